# revision 18
# baseline (speedup 1.0000x reference)
"""Trainium2 Bass kernel for the fused broadcast multiply-add:

    out[s, i, f] = x[s, i] * W[i, f] + b[i, f]

Shapes (hardcoded): x [16384, 32] f32, W [32, 256] f32, b [32, 256] f32,
out [16384, 32, 256] f32 (512 MB) -- a pure HBM-write-bound problem.

Strategy
--------
Data parallel over 8 NeuronCores: each core handles 2048 batch rows and
writes a 64 MB output shard. The kernel is output-DMA-bound; everything
else is engineered to keep the 16 SDMA engines at line rate end to end.

Compute: single-fp16 TensorE matmuls (the 2e-2 relative-error budget is
~20x looser than a single fp16 product, so no hi/lo split). Each 512-col
chunk n covers i = {2n, 2n+1} and contracts over K=4 rows at partition
base 32*(n%4) (LDWEIGHTS requires 32-aligned bases), slot s=n//4 on the
free dim:

    lhsT rows: x[i0], x[i1], 1, 0       rhs rows: W[i0]|0, 0|W[i1], b, 0

Inputs load as compact per-group DMAs (0.26 MB total vs 2.5 MB for the
hi/lo baseline), so the store stream is essentially output bytes only.

Store-stripe rotation: traces show that on ~3 of 8 cores exactly one
SDMA engine runs ~15% slower for the whole kernel (HBM address-stripe
contention; which engine varies by core). HWDGE deals a transfer's
descriptors to the 16 SDMA engines by OUTERMOST-AP-dim index mod 16
(measured across three kernel variants), so with the natural store AP
[[rows 128],[cols]] engine e serves output rows == e (mod 16) for the
entire kernel, and a slow row-stripe pins to one engine -- a ~22 us
serial tail while the other 15 idle. Fix: consecutive tile PAIRS write
a 256-row DRAM span with their rows interleaved at granularity
m in {8,4,2}: the dest AP [[2m*NF, 128/m],[NF, m],[1, C]] is
non-mergeable (survives AP optimization), keeps the outer dim >= 16
(even dealing), and binds engine e to row-blocks [2m*e + g*m, ...]
instead of the mod-16 residue. Cycling m (plus natural) per pair gives
4 distinct engine->stripe patterns, so a slow stripe is time-shared
across engines instead of pinned. The host gathers the batch columns of
the activation tile to match, so the output lands in natural order.
"""

import numpy as np

import concourse.bass as bass
import concourse.bacc as bacc
import concourse.mybir as mybir
import concourse.tile as tile
from concourse import bass_utils

BS, DEMO, FEAT = 16384, 32, 256
NCORES = 8
BSH = BS // NCORES        # 2048 batch rows per core
PT = 128                  # batch rows per matmul tile (out partitions)
NTILES = BSH // PT        # 16
NF = DEMO * FEAT          # 8192 output columns
NCHUNK = 512              # fp32 columns per PSUM bank / matmul
NCH = NF // NCHUNK        # 16 chunks (each covers two i values)
NSLOT = NCH // 4          # 4 free-dim slots per row-group
KR = 4                    # lhsT rows per chunk: x[i0], x[i1], ones, zeros

# Per-tile-pair interleave granularity, cycled over pairs: None =
# natural (engine e <- rows == e mod 16), m = pair-interleaved at m rows
# (engine e <- rows [2m*e + g*m, +m) of the 256-row span). All keep the
# outer AP dim >= 16 so descriptors deal evenly across all 16 engines.
PAT = (None, 8, 4, 2)


def _rowmap(T, p):
    """Global output row written by PSUM partition p of batch tile T."""
    j, g = T // 2, T % 2
    m = PAT[j % len(PAT)]
    if m is None:
        return T * PT + p
    return 2 * PT * j + (p // m) * (2 * m) + g * m + (p % m)

_cache: dict = {}


def _build():
    nc = bacc.Bacc("TRN2", target_bir_lowering=False, debug=False)

    # Compact combined input: row 4r+k of comb_d is row k of group r;
    # cols [0, NSLOT*BSH) = activations, rest = W/bias rhs slices.
    CW = NSLOT * BSH + NSLOT * NCHUNK
    comb_d = nc.dram_tensor(
        "comb", (4 * KR, CW), mybir.dt.float16, kind="ExternalInput"
    )
    out_d = nc.dram_tensor("out", (BSH, NF), mybir.dt.float32, kind="ExternalOutput")
    WOFF = NSLOT * BSH

    with tile.TileContext(nc) as tc:
        with (
            tc.tile_pool(name="const", bufs=1) as cpool,
            tc.tile_pool(name="opool", bufs=4) as opool,
            tc.tile_pool(name="psum", bufs=4, space=bass.MemorySpace.PSUM) as psum,
        ):
            comb_t = cpool.tile([128, CW], mybir.dt.float16)
            for r in range(4):
                nc.sync.dma_start(
                    comb_t[32 * r:32 * r + KR, :], comb_d.ap()[KR * r:KR * (r + 1), :]
                )

            for t in range(NTILES):
                o_t = opool.tile([PT, NF], mybir.dt.float32)
                j, gg = t // 2, t % 2
                m = PAT[j % len(PAT)]
                for g in range(8):  # copy groups of 1024 cols (2 chunks)
                    acc = psum.tile([PT, 2 * NCHUNK], mybir.dt.float32)
                    for h in range(2):
                        n = 2 * g + h
                        r, s = n % 4, n // 4
                        nc.tensor.matmul(
                            acc[:, h * NCHUNK:(h + 1) * NCHUNK],
                            comb_t[32 * r:32 * r + KR,
                                   s * BSH + t * PT: s * BSH + (t + 1) * PT],
                            comb_t[32 * r:32 * r + KR,
                                   WOFF + s * NCHUNK: WOFF + (s + 1) * NCHUNK],
                            start=True,
                            stop=True,
                            tile_position=(32 * r, 0),
                        )
                    dst = o_t[:, g * 1024:(g + 1) * 1024]
                    if g % 2 == 0:
                        nc.vector.tensor_copy(dst, acc[:])
                    else:
                        nc.scalar.copy(dst, acc[:])
                    if g in (3, 7):  # 2 MB half-tile stores (16 KB descs)
                        lo, hi = (g - 3) * 1024, (g + 1) * 1024
                        if m is None:
                            dest = out_d.ap()[t * PT:(t + 1) * PT, lo:hi]
                        else:
                            span = out_d.ap()[2 * PT * j:2 * PT * (j + 1), lo:hi]
                            dest = span.rearrange(
                                "(a g b) c -> g a b c", g=2, b=m
                            )[gg]
                        nc.sync.dma_start(dest, o_t[:, lo:hi])

    nc.compile()
    return nc


def _get_nc():
    if "nc" not in _cache:
        _cache["nc"] = _build()
    return _cache["nc"]


def _prep_wbp(W, b):
    W16 = np.asarray(W, dtype=np.float32).astype(np.float16)
    b16 = np.asarray(b, dtype=np.float32).astype(np.float16)
    wbp = np.zeros((4 * KR, NSLOT * NCHUNK), dtype=np.float16)
    for n in range(NCH):
        r, s = n % 4, n // 4
        p0, c0 = KR * r, s * NCHUNK
        wbp[p0 + 0, c0:c0 + FEAT] = W16[2 * n]
        wbp[p0 + 1, c0 + FEAT:c0 + 2 * FEAT] = W16[2 * n + 1]
        wbp[p0 + 2, c0:c0 + FEAT] = b16[2 * n]
        wbp[p0 + 2, c0 + FEAT:c0 + 2 * FEAT] = b16[2 * n + 1]
    return wbp


def _prep_xap(x_shard):
    """[BSH, DEMO] f32 -> [16, NSLOT*BSH] fp16 lhsT layout, with batch
    columns gathered per tile to match the store-side row interleave."""
    x16 = np.asarray(x_shard, dtype=np.float32).astype(np.float16)
    xp = np.empty_like(x16)
    p = np.arange(PT)
    for t in range(NTILES):
        xp[t * PT:(t + 1) * PT] = x16[_rowmap(t, p)]
    xT = np.ascontiguousarray(xp.T)  # [DEMO, BSH]
    xap = np.zeros((4 * KR, NSLOT * BSH), dtype=np.float16)
    for n in range(NCH):
        r, s = n % 4, n // 4
        p0 = KR * r
        xs = slice(s * BSH, (s + 1) * BSH)
        xap[p0 + 0, xs] = xT[2 * n]
        xap[p0 + 1, xs] = xT[2 * n + 1]
        xap[p0 + 2, xs] = 1.0
    return xap


def _in_maps(x, W, b):
    wbp = _prep_wbp(W, b)
    x = np.asarray(x, dtype=np.float32)
    maps = []
    for c in range(NCORES):
        xap = _prep_xap(x[c * BSH:(c + 1) * BSH])
        comb = np.concatenate([xap, wbp], axis=1)
        maps.append({"comb": comb})
    return maps


def run_shards(x, W, b, **spmd_kwargs):
    """Run the SPMD kernel; returns the BassKernelResults (for profiling)."""
    nc = _get_nc()
    return bass_utils.run_bass_kernel_spmd(
        nc, _in_maps(x, W, b), core_ids=list(range(NCORES)), **spmd_kwargs
    )


def kernel(x, W, b):
    res = run_shards(x, W, b)
    out = np.concatenate([res.results[c]["out"] for c in range(NCORES)], axis=0)
    return out.reshape(BS, DEMO, FEAT)


# revision 19
# speedup vs baseline: 1.0676x; 1.0676x over previous
"""Trainium2 Bass kernel for the fused broadcast multiply-add:

    out[s, i, f] = x[s, i] * W[i, f] + b[i, f]

Shapes (hardcoded): x [16384, 32] f32, W [32, 256] f32, b [32, 256] f32,
out [16384, 32, 256] f32 (512 MB) -- a pure HBM-write-bound problem.

Strategy
--------
Data parallel over 8 NeuronCores: each core handles 2048 batch rows and
writes a 64 MB output shard (~150-180 us at the measured 360-427 GB/s
per-core store bandwidth).

On each core everything is folded into TensorE matmuls. Each 512-column
output chunk n covers i = {2n, 2n+1} only, so its contraction needs just
K=8 rows (fp16 hi/lo split of x and W for full-rate PE with ~fp32
accuracy, bias via ones-rows):

    rows: x_hi[i0], x_hi[i1], x_hi[i0], x_hi[i1], x_lo[i0], x_lo[i1], 1, 1
    rhs:  W_hi[i0]|0, 0|W_hi[i1], W_lo[i0]|0, 0|W_lo[i1],
          W_hi[i0]|0, 0|W_hi[i1], b_hi, b_lo

(x*W = x_hi*W_hi + x_hi*W_lo + x_lo*W_hi; the dropped x_lo*W_lo term is
~2^-21 relative.) Consecutive chunks rotate tile_position across the four
32-row PE groups, so each matmul's LDWEIGHTS targets rows disjoint from
the in-flight matmul and the PE pipelines back-to-back instead of paying
the isolated fill+drain per instruction.

The xap activation tensor loads as four per-slot DMAs so each chunk's
matmuls only wait for their own slot (better startup overlap than one
monolithic load -- measured). PSUM accumulates fp32; VectorE/ScalarE
alternate on [128,1024] PSUM->SBUF copies; the sync-engine HWDGE streams
2 MB half-tiles to HBM. PE and the copy engines run well under the DMA
roofline, so the kernel is output-DMA-bound as the memory target_regime
intends.
"""

import numpy as np

import concourse.bass as bass
import concourse.bacc as bacc
import concourse.mybir as mybir
import concourse.tile as tile
from concourse import bass_utils

BS, DEMO, FEAT = 16384, 32, 256
NCORES = 8
BSH = BS // NCORES        # 2048 batch rows per core
PT = 128                  # batch rows per matmul tile (out partitions)
NTILES = BSH // PT        # 16
NF = DEMO * FEAT          # 8192 output columns
NCHUNK = 512              # fp32 columns per PSUM bank / matmul
NCH = NF // NCHUNK        # 16 chunks (each covers two i values)
NSLOT = NCH // 4          # 4 free-dim slots per row-group

_cache: dict = {}


def _build():
    nc = bacc.Bacc("TRN2", target_bir_lowering=False, debug=False)

    # xap: [128, NSLOT*BSH] fp16 -- row-group r holds the 8 lhsT rows for
    # chunks n with n%4==r, at free offset (n//4)*BSH.
    # wbp: [128, NSLOT*NCHUNK] fp16 -- same layout for the rhs slices.
    xap_d = nc.dram_tensor(
        "xap", (128, NSLOT * BSH), mybir.dt.float16, kind="ExternalInput"
    )
    wbp_d = nc.dram_tensor(
        "wbp", (128, NSLOT * NCHUNK), mybir.dt.float16, kind="ExternalInput"
    )
    out_d = nc.dram_tensor("out", (BSH, NF), mybir.dt.float32, kind="ExternalOutput")

    with tile.TileContext(nc) as tc:
        with (
            tc.tile_pool(name="const", bufs=1) as cpool,
            tc.tile_pool(name="opool", bufs=3) as opool,
            tc.tile_pool(name="psum", bufs=4, space=bass.MemorySpace.PSUM) as psum,
        ):
            wbp_t = cpool.tile([128, NSLOT * NCHUNK], mybir.dt.float16)
            xap_t = cpool.tile([128, NSLOT * BSH], mybir.dt.float16)
            nc.sync.dma_start(wbp_t[:], wbp_d.ap()[:])
            # split the xap load by slot so the first chunks start early
            for s in range(NSLOT):
                nc.sync.dma_start(
                    xap_t[:, s * BSH:(s + 1) * BSH],
                    xap_d.ap()[:, s * BSH:(s + 1) * BSH],
                )

            for t in range(NTILES):
                o_t = opool.tile([PT, NF], mybir.dt.float32)
                for g in range(8):  # copy groups of 1024 cols (2 chunks)
                    acc = psum.tile([PT, 2 * NCHUNK], mybir.dt.float32)
                    for h in range(2):
                        n = 2 * g + h
                        r, s = n % 4, n // 4
                        nc.tensor.matmul(
                            acc[:, h * NCHUNK:(h + 1) * NCHUNK],
                            xap_t[32 * r:32 * r + 8,
                                  s * BSH + t * PT: s * BSH + (t + 1) * PT],
                            wbp_t[32 * r:32 * r + 8,
                                  s * NCHUNK:(s + 1) * NCHUNK],
                            start=True,
                            stop=True,
                            tile_position=(32 * r, 0),
                        )
                    dst = o_t[:, g * 1024:(g + 1) * 1024]
                    if g % 2 == 0:
                        nc.vector.tensor_copy(dst, acc[:])
                    else:
                        nc.scalar.copy(dst, acc[:])
                    if g in (3, 7):  # 2 MB half-tile stores
                        lo, hi = (g - 3) * 1024, (g + 1) * 1024
                        nc.sync.dma_start(
                            out_d.ap()[t * PT:(t + 1) * PT, lo:hi],
                            o_t[:, lo:hi],
                        )

    nc.compile()
    return nc


def _get_nc():
    if "nc" not in _cache:
        _cache["nc"] = _build()
    return _cache["nc"]


def _prep(x, W, b):
    """Host-side layout prep: fp16 hi/lo split into row-group layout."""
    x = np.asarray(x, dtype=np.float32)
    W = np.asarray(W, dtype=np.float32)
    b = np.asarray(b, dtype=np.float32)

    xT = np.ascontiguousarray(x.T)                       # [DEMO, BS]
    x_hi = xT.astype(np.float16)
    x_lo = (xT - x_hi.astype(np.float32)).astype(np.float16)
    W_hi = W.astype(np.float16)
    W_lo = (W - W_hi.astype(np.float32)).astype(np.float16)
    b_hi = b.astype(np.float16)
    b_lo = (b - b_hi.astype(np.float32)).astype(np.float16)

    xap = np.zeros((128, NSLOT * BS), dtype=np.float16)
    wbp = np.zeros((128, NSLOT * NCHUNK), dtype=np.float16)
    for n in range(NCH):
        r, s = n % 4, n // 4
        i0, i1 = 2 * n, 2 * n + 1
        p = 32 * r
        xs = slice(s * BS, (s + 1) * BS)
        xap[p + 0, xs] = x_hi[i0]
        xap[p + 1, xs] = x_hi[i1]
        xap[p + 2, xs] = x_hi[i0]
        xap[p + 3, xs] = x_hi[i1]
        xap[p + 4, xs] = x_lo[i0]
        xap[p + 5, xs] = x_lo[i1]
        xap[p + 6, xs] = 1.0
        xap[p + 7, xs] = 1.0

        c0 = s * NCHUNK
        wbp[p + 0, c0:c0 + FEAT] = W_hi[i0]
        wbp[p + 1, c0 + FEAT:c0 + 2 * FEAT] = W_hi[i1]
        wbp[p + 2, c0:c0 + FEAT] = W_lo[i0]
        wbp[p + 3, c0 + FEAT:c0 + 2 * FEAT] = W_lo[i1]
        wbp[p + 4, c0:c0 + FEAT] = W_hi[i0]
        wbp[p + 5, c0 + FEAT:c0 + 2 * FEAT] = W_hi[i1]
        wbp[p + 6, c0:c0 + FEAT] = b_hi[i0]
        wbp[p + 6, c0 + FEAT:c0 + 2 * FEAT] = b_hi[i1]
        wbp[p + 7, c0:c0 + FEAT] = b_lo[i0]
        wbp[p + 7, c0 + FEAT:c0 + 2 * FEAT] = b_lo[i1]
    return xap, wbp


def _in_maps(x, W, b):
    xap, wbp = _prep(x, W, b)
    maps = []
    for c in range(NCORES):
        # per-core xap shard: batch columns c*BSH:(c+1)*BSH of each slot
        shard = np.empty((128, NSLOT * BSH), dtype=np.float16)
        for s in range(NSLOT):
            shard[:, s * BSH:(s + 1) * BSH] = (
                xap[:, s * BS + c * BSH: s * BS + (c + 1) * BSH]
            )
        maps.append({"xap": shard, "wbp": wbp})
    return maps


def run_shards(x, W, b, **spmd_kwargs):
    """Run the SPMD kernel; returns the BassKernelResults (for profiling)."""
    nc = _get_nc()
    return bass_utils.run_bass_kernel_spmd(
        nc, _in_maps(x, W, b), core_ids=list(range(NCORES)), **spmd_kwargs
    )


def kernel(x, W, b):
    res = run_shards(x, W, b)
    out = np.concatenate([res.results[c]["out"] for c in range(NCORES)], axis=0)
    return out.reshape(BS, DEMO, FEAT)



# revision 23
# speedup vs baseline: 1.0765x; 1.0084x over previous
"""Trainium2 Bass kernel for the fused broadcast multiply-add:

    out[s, i, f] = x[s, i] * W[i, f] + b[i, f]

Shapes (hardcoded): x [16384, 32] f32, W [32, 256] f32, b [32, 256] f32,
out [16384, 32, 256] f32 (512 MB) -- a pure HBM-write-bound problem.

Strategy
--------
Data parallel over 8 NeuronCores: each core handles 2048 batch rows and
writes a 64 MB output shard (~150-180 us at the measured 360-427 GB/s
per-core store bandwidth).

On each core everything is folded into TensorE matmuls. Each 512-column
output chunk n covers i = {2n, 2n+1} only, so its contraction needs just
K=8 rows (fp16 hi/lo split of x and W for full-rate PE with ~fp32
accuracy, bias via ones-rows):

    rows: x_hi[i0], x_hi[i1], x_hi[i0], x_hi[i1], x_lo[i0], x_lo[i1], 1, 1
    rhs:  W_hi[i0]|0, 0|W_hi[i1], W_lo[i0]|0, 0|W_lo[i1],
          W_hi[i0]|0, 0|W_hi[i1], b_hi, b_lo

(x*W = x_hi*W_hi + x_hi*W_lo + x_lo*W_hi; the dropped x_lo*W_lo term is
~2^-21 relative.) Consecutive chunks rotate tile_position across the four
32-row PE groups, so each matmul's LDWEIGHTS targets rows disjoint from
the in-flight matmul and the PE pipelines back-to-back instead of paying
the isolated fill+drain per instruction.

The xap activation tensor loads as four per-slot DMAs so each chunk's
matmuls only wait for their own slot (better startup overlap than one
monolithic load -- measured). PSUM accumulates fp32; VectorE/ScalarE
alternate on [128,1024] PSUM->SBUF copies; the sync-engine HWDGE streams
2 MB half-tiles to HBM. PE and the copy engines run well under the DMA
roofline, so the kernel is output-DMA-bound as the memory target_regime
intends.
"""

import numpy as np

import concourse.bass as bass
import concourse.bacc as bacc
import concourse.mybir as mybir
import concourse.tile as tile
from concourse import bass_utils

BS, DEMO, FEAT = 16384, 32, 256
NCORES = 8
BSH = BS // NCORES        # 2048 batch rows per core
PT = 128                  # batch rows per matmul tile (out partitions)
NTILES = BSH // PT        # 16
NF = DEMO * FEAT          # 8192 output columns
NCHUNK = 512              # fp32 columns per PSUM bank / matmul
NCH = NF // NCHUNK        # 16 chunks (each covers two i values)
NSLOT = NCH // 4          # 4 free-dim slots per row-group

_cache: dict = {}


def _build():
    nc = bacc.Bacc("TRN2", target_bir_lowering=False, debug=False)

    # comb: [128, NSLOT*BSH + NSLOT*NCHUNK] fp16 -- row-group r holds the
    # 8 lhsT rows for chunks n with n%4==r; cols [0, NSLOT*BSH) are the
    # activation slots, the rest the W/bias rhs slots. One monolithic
    # load (20 KB descriptors, evenly dealt) instead of 5 serial 4 KB-desc
    # DMAs: the first matmul gates on the whole load either way (shared
    # DMA-completion lane), so fewer/bigger transfers start compute sooner.
    CW = NSLOT * BSH + NSLOT * NCHUNK
    WOFF = NSLOT * BSH
    comb_d = nc.dram_tensor("comb", (128, CW), mybir.dt.float16, kind="ExternalInput")
    out_d = nc.dram_tensor("out", (BSH, NF), mybir.dt.float32, kind="ExternalOutput")

    with tile.TileContext(nc) as tc:
        with (
            tc.tile_pool(name="const", bufs=1) as cpool,
            tc.tile_pool(name="opool", bufs=4) as opool,
            tc.tile_pool(name="psum", bufs=4, space=bass.MemorySpace.PSUM) as psum,
        ):
            comb_t = cpool.tile([128, CW], mybir.dt.float16)
            nc.sync.dma_start(comb_t[:], comb_d.ap()[:])

            for t in range(NTILES):
                o_t = opool.tile([PT, NF], mybir.dt.float32)
                for g in range(8):  # copy groups of 1024 cols (2 chunks)
                    acc = psum.tile([PT, 2 * NCHUNK], mybir.dt.float32)
                    for h in range(2):
                        n = 2 * g + h
                        r, s = n % 4, n // 4
                        nc.tensor.matmul(
                            acc[:, h * NCHUNK:(h + 1) * NCHUNK],
                            comb_t[32 * r:32 * r + 8,
                                   s * BSH + t * PT: s * BSH + (t + 1) * PT],
                            comb_t[32 * r:32 * r + 8,
                                   WOFF + s * NCHUNK: WOFF + (s + 1) * NCHUNK],
                            start=True,
                            stop=True,
                            tile_position=(32 * r, 0),
                        )
                    dst = o_t[:, g * 1024:(g + 1) * 1024]
                    if g % 2 == 0:
                        nc.vector.tensor_copy(dst, acc[:])
                    else:
                        nc.scalar.copy(dst, acc[:])
                    if g in (3, 7):  # 2 MB half-tile stores
                        lo, hi = (g - 3) * 1024, (g + 1) * 1024
                        nc.sync.dma_start(
                            out_d.ap()[t * PT:(t + 1) * PT, lo:hi],
                            o_t[:, lo:hi],
                        )

    nc.compile()
    return nc


def _get_nc():
    if "nc" not in _cache:
        _cache["nc"] = _build()
    return _cache["nc"]


def _prep(x, W, b):
    """Host-side layout prep: fp16 hi/lo split into row-group layout."""
    x = np.asarray(x, dtype=np.float32)
    W = np.asarray(W, dtype=np.float32)
    b = np.asarray(b, dtype=np.float32)

    xT = np.ascontiguousarray(x.T)                       # [DEMO, BS]
    x_hi = xT.astype(np.float16)
    x_lo = (xT - x_hi.astype(np.float32)).astype(np.float16)
    W_hi = W.astype(np.float16)
    W_lo = (W - W_hi.astype(np.float32)).astype(np.float16)
    b_hi = b.astype(np.float16)
    b_lo = (b - b_hi.astype(np.float32)).astype(np.float16)

    xap = np.zeros((128, NSLOT * BS), dtype=np.float16)
    wbp = np.zeros((128, NSLOT * NCHUNK), dtype=np.float16)
    for n in range(NCH):
        r, s = n % 4, n // 4
        i0, i1 = 2 * n, 2 * n + 1
        p = 32 * r
        xs = slice(s * BS, (s + 1) * BS)
        xap[p + 0, xs] = x_hi[i0]
        xap[p + 1, xs] = x_hi[i1]
        xap[p + 2, xs] = x_hi[i0]
        xap[p + 3, xs] = x_hi[i1]
        xap[p + 4, xs] = x_lo[i0]
        xap[p + 5, xs] = x_lo[i1]
        xap[p + 6, xs] = 1.0
        xap[p + 7, xs] = 1.0

        c0 = s * NCHUNK
        wbp[p + 0, c0:c0 + FEAT] = W_hi[i0]
        wbp[p + 1, c0 + FEAT:c0 + 2 * FEAT] = W_hi[i1]
        wbp[p + 2, c0:c0 + FEAT] = W_lo[i0]
        wbp[p + 3, c0 + FEAT:c0 + 2 * FEAT] = W_lo[i1]
        wbp[p + 4, c0:c0 + FEAT] = W_hi[i0]
        wbp[p + 5, c0 + FEAT:c0 + 2 * FEAT] = W_hi[i1]
        wbp[p + 6, c0:c0 + FEAT] = b_hi[i0]
        wbp[p + 6, c0 + FEAT:c0 + 2 * FEAT] = b_hi[i1]
        wbp[p + 7, c0:c0 + FEAT] = b_lo[i0]
        wbp[p + 7, c0 + FEAT:c0 + 2 * FEAT] = b_lo[i1]
    return xap, wbp


def _in_maps(x, W, b):
    xap, wbp = _prep(x, W, b)
    maps = []
    for c in range(NCORES):
        # per-core shard: batch columns c*BSH:(c+1)*BSH of each slot,
        # with the rhs (W/bias) slots appended on the free dim
        comb = np.empty((128, NSLOT * BSH + NSLOT * NCHUNK), dtype=np.float16)
        for s in range(NSLOT):
            comb[:, s * BSH:(s + 1) * BSH] = (
                xap[:, s * BS + c * BSH: s * BS + (c + 1) * BSH]
            )
        comb[:, NSLOT * BSH:] = wbp
        maps.append({"comb": comb})
    return maps


def run_shards(x, W, b, **spmd_kwargs):
    """Run the SPMD kernel; returns the BassKernelResults (for profiling)."""
    nc = _get_nc()
    return bass_utils.run_bass_kernel_spmd(
        nc, _in_maps(x, W, b), core_ids=list(range(NCORES)), **spmd_kwargs
    )


def kernel(x, W, b):
    res = run_shards(x, W, b)
    out = np.concatenate([res.results[c]["out"] for c in range(NCORES)], axis=0)
    return out.reshape(BS, DEMO, FEAT)

